# revision 11
# baseline (speedup 1.0000x reference)
"""DGP loss kernel for Trainium2, 8 NeuronCores, pure data parallel.

Math (algebraically identical to the reference):
  - Per-feature-cell masked lower-median M[i,j] over the aligned 4x4 depth
    block. Valid <=> d in [MIN_D, DD_THR). Median selected from a bitonic
    sort of the 16 keys (-d valid / +BIG invalid); k-dependent index picked
    with indicators [k>=t] <=> sorted_key[t-1] < 0 (no separate k count).
  - seg term: ||sf_c - sf_n||^2 = 2 - 2*dot  =>  per-pair term
    exp(2*dot-2) * exp(-|M_c - M_n|); denominator pairs (M_c>0)&(M_n>0).
  - SYMMETRY: term(x, x+o) == term(x+o, x), so only the 12 lex-positive
    offsets are computed; a host-built weight tensor w[p,o,c] in {0,1,2}
    supplies pair multiplicities (2 interior, 1 on global border strips,
    0 on junk lanes) and band ownership.

Sharding: 8 cores = 2 images x 4 column bands (63 anchor cols each + 2 halo
= 67-col block). Anchor partitions p=0..127 <-> feature rows r=p-1.

Schedule (from trace analysis): correlation (DVE prods -> PE ones-reduce ->
scalar ACT exp-evac -> DMA regroup) is declared FIRST so the PE/scalar
pipeline fills early; the depth sort runs on DVE concurrently with the
scalar evacuation train; dda/vm/combine form the tail. PE warmup matmuls
keep the HAM clock-gate open during the prologue. Depth is DMA'd directly
into sort-plane layout (strided descriptors) to avoid a 5us DVE scatter.
"""

from contextlib import ExitStack

import numpy as np
import ml_dtypes

import concourse.bass as bass
import concourse.mybir as mybir
import concourse.tile as tile
from concourse import bass_utils
from concourse.alu_op_type import AluOpType

F32 = mybir.dt.float32
BF16 = mybir.dt.bfloat16
AF = mybir.ActivationFunctionType
AX = mybir.AxisListType

EPS_FN = 1e-8
BIG = 3.0e38
MIN_D, MAX_D = 1.0, 100.0
NSCALE = 1.0 / (1.0 / MIN_D - 1.0 / MAX_D)          # 1/0.99
NBIAS = -(1.0 / MAX_D) * NSCALE                      # -0.01/0.99
# n(d) > EPS_FN  <=>  d < DD_THR (n is monotone decreasing)
DD_THR = float(1.0 / (1.0 / MAX_D + EPS_FN / NSCALE))

# Geometry (per core).
JBAND = 63          # anchor cols owned per band
JCOLS = 67          # feature cols loaded (63 + 2 halo each side)
JP = 72             # padded row stride (data at cols 2..68, zero pads around)
JW = 68             # correlation width (67 anchors + 1 junk col, even)
DCOLS = 4 * JCOLS   # depth cols loaded
NROW = 36           # feature rows per strip (32 anchors + halo)
NCEN = 32           # anchor rows per strip
NP = NCEN * JW      # per-offset dot count = 2176
NF = NROW * JP      # strip free size = 2592

# 12 lex-positive offsets (du, dv): pairs (x, x + o)
OFFS = [(0, 1), (0, 2),
        (1, -2), (1, -1), (1, 0), (1, 1), (1, 2),
        (2, -2), (2, -1), (2, 0), (2, 1), (2, 2)]
NO = len(OFFS)


def _oddeven_merge_sort_layers(n):
    """Batcher odd-even mergesort compare-exchange pairs, grouped by layer."""
    layers = []
    p = 1
    while p < n:
        k = p
        while k >= 1:
            layer = []
            for j in range(k % p, n - k, 2 * k):
                for i in range(0, min(k, n - j - k)):
                    if (i + j) // (p * 2) == (i + j + k) // (p * 2):
                        layer.append((i + j, i + j + k))
            layers.append(layer)
            k //= 2
        p *= 2
    return layers


def _group_runs(pairs):
    """Group CE pairs (a, a+d) of one layer into (a0, step, count, d) runs."""
    by_d = {}
    for a, b in pairs:
        by_d.setdefault(b - a, []).append(a)
    runs = []
    for d, alist in sorted(by_d.items()):
        alist = sorted(alist)
        i = 0
        while i < len(alist):
            j = i + 1
            step = None
            while j < len(alist):
                s = alist[j] - alist[j - 1]
                if step is None:
                    step = s
                elif s != step:
                    break
                j += 1
            cnt = j - i
            runs.append((alist[i], step if cnt > 1 else 1, cnt, d))
            i = j
    return runs


def _ap_runs(idxs):
    """Group a sorted index list into (start, step, count) arithmetic runs."""
    runs = []
    i = 0
    while i < len(idxs):
        j = i + 1
        step = None
        while j < len(idxs):
            s = idxs[j] - idxs[j - 1]
            if step is None:
                step = s
            elif s != step:
                break
            j += 1
        cnt = j - i
        runs.append((idxs[i], step if cnt > 1 else 1, cnt))
        i = j
    return runs


def _planes(t, start, step, count):
    """AP over plane dim of a [128, NPLANES, W] tile."""
    if count == 1:
        return t[:, start : start + 1, :]
    return t[:, start : start + (count - 1) * step + 1 : step, :]


def _split_excess_waits(nc, max_waits=1):
    """Walrus rejects instructions carrying more than one sem-wait; move the
    excess onto standalone EventSemaphore waits."""
    for f in nc.m.functions:
        for blk in f.blocks:
            new_insts = []
            for inst in blk.instructions:
                si = inst.sync_info
                if si is not None and si.on_wait and len(si.on_wait) > max_waits:
                    waits = list(si.on_wait)
                    excess, keep = waits[:-max_waits], waits[-max_waits:]
                    idx = 0
                    while excess:
                        chunk, excess = excess[:max_waits], excess[max_waits:]
                        new_insts.append(
                            mybir.InstEventSemaphore(
                                name=f"{inst.name}-wsplit{idx}",
                                engine=inst.engine,
                                ins=[],
                                outs=[],
                                sync_info=mybir.SyncInfo(on_wait=chunk, on_update=[]),
                            )
                        )
                        idx += 1
                    si.on_wait = keep
                new_insts.append(inst)
            blk.instructions[:] = new_insts


def _act_rsqrt(nc, out, in_, bias_ap):
    """Raw Rsqrt InstActivation: out = 1/sqrt(in_ + bias)."""
    act = nc.scalar
    inputs = [
        act.lower_ap(in_),
        act.lower_ap(bias_ap),
        mybir.ImmediateValue(dtype=mybir.dt.float32, value=1.0),
        mybir.ImmediateValue(dtype=mybir.dt.float32, value=0.0),
    ]
    return act.add_instruction(
        mybir.InstActivation(
            name=nc.get_next_instruction_name(),
            func=AF.Rsqrt,
            ins=inputs,
            outs=[act.lower_ap(out)],
        )
    )


def _build_core_program(split_waits=True):
    nc = bass.Bass("TRN2", target_bir_lowering=False, debug=False)
    dep = nc.dram_tensor("dep", [512, DCOLS], F32, kind="ExternalInput")
    sf = nc.dram_tensor("sf", [32, 128, JCOLS], F32, kind="ExternalInput")
    wt = nc.dram_tensor("w", [128, NO, JW], BF16, kind="ExternalInput")
    obc = nc.dram_tensor("obc", [4, 128], F32, kind="ExternalInput")
    out = nc.dram_tensor("out", [128, 2], F32, kind="ExternalOutput")

    with tile.TileContext(nc) as tc, ExitStack() as ctx:
        persist = ctx.enter_context(tc.tile_pool(name="persist", bufs=1))
        work = ctx.enter_context(tc.tile_pool(name="work", bufs=1))
        prods = ctx.enter_context(tc.tile_pool(name="prods", bufs=3))

        v = nc.vector
        act = nc.scalar

        # ---------------- input DMAs ----------------
        # depth straight into sort-plane layout: plane (r,s) of the 4x4 block
        # at feature cell (row p, block col j)
        dep_scat = work.tile([128, 16, JW], F32)
        dep_4d = dep.ap().rearrange("(i r) (j s) -> i r j s", r=4, s=4)
        for r4 in range(4):
            for s4 in range(4):
                nc.sync.dma_start(
                    out=dep_scat[:, 4 * r4 + s4, 0:JCOLS],
                    in_=dep_4d[:, r4, :, s4],
                )

        sf_strip = persist.tile([128, NROW, JP], F32)
        v.memset(sf_strip[:, :, 0:2], 0.0)
        v.memset(sf_strip[:, :, JP - 3 : JP], 0.0)
        v.memset(sf_strip[0:32, 0:3, :], 0.0)
        v.memset(sf_strip[96:128, NROW - 1 : NROW, :], 0.0)
        # strip s holds feature rows 32s-3 .. 32s+32 at planes 0..35
        strip_rows = [(0, 33, 3), (29, 65, 0), (61, 97, 0), (93, 128, 0)]
        for s, (r0, r1, pl0) in enumerate(strip_rows):
            nc.sync.dma_start(
                out=sf_strip[32 * s : 32 * (s + 1), pl0 : pl0 + (r1 - r0), 2 : 2 + JCOLS],
                in_=sf.ap()[:, r0:r1, :],
            )

        w_t = persist.tile([128, NO, JW], BF16)
        nc.sync.dma_start(out=w_t, in_=wt.ap())

        # constants
        ones4f = persist.tile([128, 4], F32)
        v.memset(ones4f, 0.0)
        ones4b = persist.tile([128, 4], BF16)
        v.memset(ones4b, 0.0)
        for s in range(4):
            v.memset(ones4f[32 * s : 32 * (s + 1), s : s + 1], 1.0)
            v.memset(ones4b[32 * s : 32 * (s + 1), s : s + 1], 1.0)
        ones_bc = persist.tile([4, 128], F32)
        nc.sync.dma_start(out=ones_bc, in_=obc.ap())
        eps_b = persist.tile([4, 1], F32)
        v.memset(eps_b, 1e-24)
        bias_m2 = persist.tile([4, 1], F32)
        v.memset(bias_m2, -2.0)
        bias_z = persist.tile([128, 1], F32)
        v.memset(bias_z, 0.0)

        # ---------------- seg normalization ----------------
        f2 = work.tile([128, NROW, JP], F32)
        act.activation(f2, sf_strip, AF.Square)
        rinv = work.tile([4, NF], F32)
        with tc.tile_pool(name="psnrm", bufs=1, space="PSUM") as psnrm:
            nrm2_ps = psnrm.tile([4, NF], F32, tag="nrm2")
            for c0 in range(0, NF, 512):
                cw = min(512, NF - c0)
                nc.tensor.matmul(
                    nrm2_ps[:, c0 : c0 + cw],
                    ones4f,
                    f2.rearrange("p a b -> p (a b)")[:, c0 : c0 + cw],
                )
            _act_rsqrt(nc, rinv, nrm2_ps, eps_b)

        sfb = persist.tile([128, NROW, JP], BF16)
        with tc.tile_pool(name="psbc", bufs=1, space="PSUM") as psbc:
            bc_ps = psbc.tile([128, NF], F32, tag="bcast")
            for c0 in range(0, NF, 512):
                cw = min(512, NF - c0)
                nc.tensor.matmul(
                    bc_ps[:, c0 : c0 + cw], ones_bc, rinv[:, c0 : c0 + cw]
                )
            v.tensor_tensor(
                sfb.rearrange("p a b -> p (a b)"),
                sf_strip.rearrange("p a b -> p (a b)"),
                bc_ps,
                op=AluOpType.mult,
            )
        # 1-col-left-shifted copy for odd dv offsets (keeps bf16 4B alignment)
        sfb1 = persist.tile([128, NROW, JP], BF16)
        v.memset(sfb1[:, :, JP - 1 : JP], 0.0)
        nc.sync.dma_start(
            out=sfb1.rearrange("p a b -> p (a b)")[:, 0 : NF - 1],
            in_=sfb.rearrange("p a b -> p (a b)")[:, 1:NF],
        )

        # ---------------- PE warmup (keeps HAM clock-gate open) -------------
        warm_sink = work.tile([4, 1], F32)
        with tc.tile_pool(name="pswarm", bufs=1, space="PSUM") as pswarm:
            wm_ps = pswarm.tile([4, 512], F32, tag="warm")
            for i in range(24):
                nc.tensor.matmul(
                    wm_ps, ones4f, f2.rearrange("p a b -> p (a b)")[:, 0:512]
                )
            act.activation(warm_sink, wm_ps[:, 0:1], AF.Copy, bias=0.0, scale=1.0)

        # ---------------- depth preproc + sort keys (DVE) ----------------
        # depn = -d; valid <=> depn <= -1 AND depn > -THR
        depn = work.tile([128, 16, JW], BF16)
        v.tensor_scalar(depn, dep_scat, -1.0, None, op0=AluOpType.mult)
        le1 = work.tile([128, 16, JW], BF16)
        v.tensor_scalar(le1, depn, -MIN_D, None, op0=AluOpType.is_le)
        gtT = work.tile([128, 16, JW], BF16)
        v.tensor_scalar(gtT, depn, -DD_THR, None, op0=AluOpType.is_gt)
        valid = work.tile([128, 16, JW], BF16)
        v.tensor_tensor(valid, le1, gtT, op=AluOpType.mult)
        dv_t = work.tile([128, 16, JW], BF16)
        v.tensor_tensor(dv_t, valid, depn, op=AluOpType.mult)
        nvb = work.tile([128, 16, JW], BF16)
        v.tensor_scalar(nvb, valid, 0.5, BIG, op0=AluOpType.is_lt, op1=AluOpType.mult)

        SA = persist.tile([128, 16, JW], BF16, tag="SA")
        SB = persist.tile([128, 16, JW], BF16, tag="SB")
        v.memset(SA, BIG)
        v.tensor_tensor(
            SA[:, :, 0:JCOLS], dv_t[:, :, 0:JCOLS], nvb[:, :, 0:JCOLS],
            op=AluOpType.add,
        )

        # ---------------- 12-offset correlation (PE + scalar pipeline) ------
        dots_rg = persist.tile([128, NO, JW], BF16)   # exp(2*dot - 2)
        psum = ctx.enter_context(tc.tile_pool(name="psum", bufs=1, space="PSUM"))
        dps = psum.tile([36, NP], F32, tag="dps")

        cen = sfb[:, 2 : 2 + NCEN, 2 : 2 + JW]
        for o, (du, dv) in enumerate(OFFS):
            if dv % 2 == 0:
                par = sfb[:, 2 + du : 2 + du + NCEN, 2 + dv : 2 + dv + JW]
            else:
                par = sfb1[:, 2 + du : 2 + du + NCEN, 1 + dv : 1 + dv + JW]
            prod = prods.tile([128, NCEN, JW], BF16, tag="prod")
            v.tensor_tensor(prod, cen, par, op=AluOpType.mult)
            q = 32 * (o % 2)
            for c0 in range(0, NP, 512):
                cw = min(512, NP - c0)
                nc.tensor.matmul(
                    dps[q : q + 4, c0 : c0 + cw],
                    ones4b,
                    prod.rearrange("p a b -> p (a b)")[:, c0 : c0 + cw],
                )
            # evacuate with fused exp(2*dots - 2) -> bf16
            dsb = prods.tile([4, NP], BF16, tag="dsb")
            act.activation(dsb, dps[q : q + 4, :], AF.Exp, bias=bias_m2, scale=2.0)
            nc.sync.dma_start(
                out=dots_rg[:, o, :], in_=dsb.rearrange("s (a c) -> s a c", a=NCEN)
            )

        # ---------------- sort (DVE, overlaps the evac train) ---------------
        bufs = [SA, SB]
        layers = _oddeven_merge_sort_layers(16)
        for li, layer in enumerate(layers):
            src, dst = bufs[li % 2], bufs[(li + 1) % 2]
            touched = set()
            for a, b in layer:
                touched.add(a)
                touched.add(b)
            for a0, astep, cnt, d in _group_runs(layer):
                lo_s = _planes(src, a0, astep, cnt)
                hi_s = _planes(src, a0 + d, astep, cnt)
                v.tensor_tensor(_planes(dst, a0, astep, cnt), lo_s, hi_s, op=AluOpType.min)
                v.tensor_tensor(
                    _planes(dst, a0 + d, astep, cnt), lo_s, hi_s, op=AluOpType.max
                )
            untouched = sorted(set(range(16)) - touched)
            for u0, ustep, ucnt in _ap_runs(untouched):
                v.tensor_copy(_planes(dst, u0, ustep, ucnt), _planes(src, u0, ustep, ucnt))
        S = bufs[len(layers) % 2]

        # ---------------- median select ----------------
        G = work.tile([128, 8, JW], BF16)
        v.tensor_scalar(G, S[:, 0:16:2, :], 0.0, None, op0=AluOpType.is_lt)
        u = work.tile([128, 8, JW], BF16)
        v.tensor_tensor(u[:, 0:7, :], G[:, 0:7, :], G[:, 1:8, :], op=AluOpType.subtract)
        v.tensor_copy(u[:, 7:8, :], G[:, 7:8, :])
        sel = work.tile([128, 8, JW], BF16)
        v.tensor_tensor(sel, S[:, 0:8, :], u, op=AluOpType.mult)
        mdneg = work.tile([128, JW], F32)
        v.reduce_sum(out=mdneg, in_=sel.rearrange("p m j -> p j m"), axis=AX.X)
        med_d = work.tile([128, JW], F32)
        v.tensor_scalar(med_d, mdneg, -1.0, 1.0, op0=AluOpType.mult, op1=AluOpType.max)
        rec_s = work.tile([128, JW], F32)
        v.reciprocal(rec_s, med_d)
        aff_s = work.tile([128, JW], F32)
        act.activation(aff_s, rec_s, AF.Copy, bias=NBIAS, scale=NSCALE)

        Kpos = persist.tile([128, JP], F32)
        v.memset(Kpos, 0.0)
        v.tensor_scalar(Kpos[:, 2 : 2 + JW], S[:, 0, :], 0.0, None, op0=AluOpType.is_lt)
        Kb = persist.tile([128, JP], BF16)
        v.tensor_copy(Kb, Kpos)
        M = persist.tile([128, JP], F32)
        v.memset(M, 0.0)
        v.tensor_tensor(M[:, 2 : 2 + JW], aff_s, Kpos[:, 2 : 2 + JW], op=AluOpType.mult)

        # row-shifted copies: Xm1[p] = X[p-1], Xp1[p] = X[p+1]
        M_m1 = persist.tile([128, JP], F32, tag="M_m1")
        v.memset(M_m1, 0.0)
        nc.sync.dma_start(out=M_m1[1:128, :], in_=M[0:127, :])
        M_p1 = persist.tile([128, JP], F32, tag="M_p1")
        v.memset(M_p1, 0.0)
        nc.sync.dma_start(out=M_p1[0:127, :], in_=M[1:128, :])
        K_m1 = persist.tile([128, JP], BF16, tag="K_m1")
        v.memset(K_m1, 0.0)
        nc.sync.dma_start(out=K_m1[1:128, :], in_=Kb[0:127, :])
        K_p1 = persist.tile([128, JP], BF16, tag="K_p1")
        v.memset(K_p1, 0.0)
        nc.sync.dma_start(out=K_p1[0:127, :], in_=Kb[1:128, :])
        M_by_du = [M_m1, M, M_p1]
        K_by_du = [K_m1, Kb, K_p1]

        # ---------------- dda / vm ----------------
        dda = persist.tile([128, NO, JW], F32)
        vm = persist.tile([128, NO, JW], BF16)
        for o, (du, dv) in enumerate(OFFS):
            v.tensor_tensor(
                dda[:, o, :],
                M_m1[:, 2 : 2 + JW],
                M_by_du[du][:, 2 + dv : 2 + dv + JW],
                op=AluOpType.subtract,
            )
            v.tensor_tensor(
                vm[:, o, :],
                K_m1[:, 2 : 2 + JW],
                K_by_du[du][:, 2 + dv : 2 + dv + JW],
                op=AluOpType.mult,
            )

        # ---------------- combine ----------------
        act.activation(dda, dda, AF.Abs)
        expdda = work.tile([128, NO, JW], BF16)
        act.activation(expdda, dda, AF.Exp, bias=bias_z, scale=-1.0)
        wexp = work.tile([128, NO, JW], BF16)
        v.tensor_tensor(wexp, expdda, w_t, op=AluOpType.mult)
        terms = work.tile([128, NO, JW], BF16)
        v.tensor_tensor(terms, dots_rg, wexp, op=AluOpType.mult)
        vmw = work.tile([128, NO, JW], BF16)
        v.tensor_tensor(vmw, vm, w_t, op=AluOpType.mult)

        numden = work.tile([128, 2], F32)
        v.reduce_sum(out=numden[:, 0:1], in_=terms, axis=AX.XY)
        v.reduce_sum(out=numden[:, 1:2], in_=vmw, axis=AX.XY)
        nc.sync.dma_start(out=out.ap(), in_=numden)

    if split_waits:
        _split_excess_waits(nc)
    return nc


def _build_weights(band):
    """w[p, o, c] in {0,1,2}: pair multiplicity for anchor (row r=p-1,
    feature col j=63*band+c), offset o."""
    p = np.arange(128)[:, None, None]
    r = p - 1
    c = np.arange(JW)[None, None, :]
    j = 63 * band + c
    du = np.array([o[0] for o in OFFS])[None, :, None]
    dv = np.array([o[1] for o in OFFS])[None, :, None]
    j0 = 63 * band + 2
    w1 = (
        (r >= 2) & (r <= 125) & (j >= 2) & (j <= 253) & (j >= j0) & (j <= j0 + 62)
    ).astype(np.float32)
    w2 = (
        (r + du >= 2) & (r + du <= 125)
        & (j + dv >= 2) & (j + dv <= 253)
        & (j + dv >= j0) & (j + dv <= j0 + 62)
    ).astype(np.float32)
    return (w1 + w2).astype(ml_dtypes.bfloat16)


_NC_CACHE = []
_W_CACHE = {}
_OBC = np.zeros((4, 128), dtype=np.float32)
for _s in range(4):
    _OBC[_s, 32 * _s : 32 * (_s + 1)] = 1.0


def make_in_maps(seg_feat, dep_true):
    in_maps = []
    for core in range(8):
        img, band = core // 4, core % 4
        j0 = JBAND * band
        if band not in _W_CACHE:
            _W_CACHE[band] = _build_weights(band)
        in_maps.append(
            {
                "dep": np.ascontiguousarray(dep_true[img, :, 4 * j0 : 4 * j0 + DCOLS]),
                "sf": np.ascontiguousarray(seg_feat[img, :, :, j0 : j0 + JCOLS]),
                "w": _W_CACHE[band],
                "obc": _OBC,
            }
        )
    return in_maps


def kernel(seg_feat: np.ndarray, dep_true: np.ndarray) -> np.ndarray:
    seg_feat = np.ascontiguousarray(seg_feat, dtype=np.float32)
    dep_true = np.ascontiguousarray(dep_true, dtype=np.float32)

    if not _NC_CACHE:
        _NC_CACHE.append(_build_core_program())
    nc = _NC_CACHE[0]

    in_maps = make_in_maps(seg_feat, dep_true)
    res = bass_utils.run_bass_kernel_spmd(nc, in_maps, core_ids=list(range(8)))
    parts = [r["out"].astype(np.float64) for r in res.results]

    loss = 0.0
    for img in range(2):
        num = sum(parts[img * 4 + b][:, 0].sum() for b in range(4))
        den = sum(parts[img * 4 + b][:, 1].sum() for b in range(4))
        loss += num / max(den, 1.0)
    return np.float32(loss / 2.0)
